# revision 7
# baseline (speedup 1.0000x reference)
"""ContactNet grasp-head kernel for 8 Trainium2 NeuronCores.

Data-parallel over points: 327680 points -> 8 shards of 40960.
Head weights are tiny and replicated to every core.

Per-core device program (Bass/Tile):
  - feat tiles are PE-transposed to [feat, pts] layout,
  - layer-1 of the 4 heads: stationary-weight matmuls, k=131 split as
    k=128 (features) + k=3 (xyz, against PE-transposed points),
  - ReLU+bias on the scalar engine straight out of PSUM,
  - layer-2: 4 accumulating matmuls with a block-diagonal packed W2
    producing one [8, Np] tile (rows: s, z1xyz, z2xyz, w),
  - PE transpose-back to [pts, 8], SoA shuffle on the vector engine,
  - grasp-frame math (normalize / orthogonalize / cross / clip) on wide
    [128, 64] slices, grasp matrices assembled and DMAd out.

Outputs that are verbatim copies of inputs (points, concatenated
features) are assembled on the host.
"""

import os
import sys

import numpy as np

sys.path.insert(0, "/opt/trn_rl_repo")

N_CORES = 8
N_TOTAL = 327680
NC_PTS = N_TOTAL // N_CORES        # 40960
GROUP_PTS = 8192                   # postprocess batch (64 chunks of 128)
N_GROUPS = NC_PTS // GROUP_PTS     # 5
SC_PTS = 512                       # matmul super-chunk
N_SC = GROUP_PTS // SC_PTS         # 16 per group
W = GROUP_PTS // 128               # 64 SoA slice width (chunks per group)
FEAT = 128
HID = 128
GRIPPER_DEPTH = 0.1034
HEAD_OFF = (0, 1, 4, 7)            # s, z1, z2, w rows in the packed [8] dim
HEAD_OD = (1, 3, 3, 1)

_CACHE = {}


def _build_nc(nc_pts=NC_PTS):
    import concourse.bacc as bacc
    import concourse.mybir as mybir
    from concourse.tile import TileContext

    f32 = mybir.dt.float32
    AF = mybir.ActivationFunctionType
    ALU = mybir.AluOpType

    n_groups = nc_pts // GROUP_PTS
    nc = bacc.Bacc("TRN2", target_bir_lowering=False, debug=False)

    feat_d = nc.dram_tensor("feat", [nc_pts, FEAT], f32, kind="ExternalInput")
    pts_d = nc.dram_tensor("pts", [nc_pts, 3], f32, kind="ExternalInput")
    w1f_d = nc.dram_tensor("w1f", [128, 512], f32, kind="ExternalInput")
    w1x_d = nc.dram_tensor("w1x", [3, 512], f32, kind="ExternalInput")
    w2p_d = nc.dram_tensor("w2p", [128, 32], f32, kind="ExternalInput")
    b1p_d = nc.dram_tensor("b1p", [128, 4], f32, kind="ExternalInput")
    b2p_d = nc.dram_tensor("b2p", [8, 1], f32, kind="ExternalInput")
    id128_d = nc.dram_tensor("id128", [128, 128], f32, kind="ExternalInput")
    id8_d = nc.dram_tensor("id8", [8, 8], f32, kind="ExternalInput")
    grasps_d = nc.dram_tensor("grasps", [nc_pts, 16], f32, kind="ExternalOutput")
    sw_d = nc.dram_tensor("sw", [nc_pts, 2], f32, kind="ExternalOutput")

    from contextlib import ExitStack

    with ExitStack() as stack:
        tc = stack.enter_context(TileContext(nc))
        pool = lambda *a, **k: stack.enter_context(tc.tile_pool(*a, **k))
        consts = pool(name="consts", bufs=1)
        feat_in = pool(name="feat_in", bufs=3)
        featT_p = pool(name="featT", bufs=3)
        pts_in = pool(name="pts_in", bufs=2)
        ptsT_p = pool(name="ptsT", bufs=2)
        hsb = pool(name="hsb", bufs=8)
        o8sb = pool(name="o8sb", bufs=3)
        soa_p = pool(name="soa", bufs=2)
        soap_p = pool(name="soap", bufs=2)
        scr = pool(name="scr", bufs=2)
        gout_p = pool(name="gout", bufs=2)
        sw_p = pool(name="swsb", bufs=2)
        ps_ft = pool(name="ps_ft", bufs=2, space="PSUM")
        ps_h = pool(name="ps_h", bufs=2, space="PSUM")
        ps_o8 = pool(name="ps_o8", bufs=1, space="PSUM")
        ps_tb = pool(name="ps_tb", bufs=2, space="PSUM")
        if True:
            w1f_s = consts.tile([128, 512], f32)
            nc.sync.dma_start(w1f_s[:], w1f_d[:])
            w1x_s = consts.tile([3, 512], f32)
            nc.sync.dma_start(w1x_s[:], w1x_d[:])
            w2_s = consts.tile([128, 32], f32)
            nc.sync.dma_start(w2_s[:], w2p_d[:])
            b1_s = consts.tile([128, 4], f32)
            nc.sync.dma_start(b1_s[:], b1p_d[:])
            b2_s = consts.tile([8, 1], f32)
            nc.sync.dma_start(b2_s[:], b2p_d[:])
            id128 = consts.tile([128, 128], f32)
            nc.sync.dma_start(id128[:], id128_d[:])
            id8 = consts.tile([8, 8], f32)
            nc.sync.dma_start(id8[:], id8_d[:])

            for g in range(n_groups):
                g0 = g * GROUP_PTS
                # ---- points: load AoS [128, (c j)] and PE-transpose ----
                ptile = pts_in.tile([128, 3 * W], f32)
                nc.sync.dma_start(
                    ptile[:].rearrange("p (c j) -> p c j", j=3),
                    pts_d[g0:g0 + GROUP_PTS, :].rearrange(
                        "(c p) j -> p c j", p=128
                    ),
                )
                ptsT3 = ptsT_p.tile([3, GROUP_PTS], f32)
                nc.sync.dma_start(
                    ptsT3[:],
                    pts_d[g0:g0 + GROUP_PTS, :].rearrange("n j -> j n"),
                )

                tb = ps_tb.tile([128, 512], f32)

                for sc in range(N_SC):
                    base = g0 + sc * SC_PTS
                    ft = feat_in.tile([128, 512], f32)
                    nc.sync.dma_start(
                        ft[:].rearrange("p (c f) -> p c f", f=128),
                        feat_d[base:base + SC_PTS, :].rearrange(
                            "(c p) f -> p c f", p=128
                        ),
                    )
                    fT_ps = ps_ft.tile([128, 512], f32)
                    for c in range(4):
                        nc.tensor.transpose(
                            fT_ps[:, c * 128:(c + 1) * 128],
                            ft[:, c * 128:(c + 1) * 128],
                            id128[:],
                        )
                    fT = featT_p.tile([128, 512], f32)
                    nc.vector.tensor_copy(fT[:], fT_ps[:])

                    hts = []
                    for h in range(4):
                        ph = ps_h.tile([128, 512], f32, tag="ph")
                        nc.tensor.matmul(
                            ph[:],
                            w1f_s[:, h * 128:(h + 1) * 128],
                            fT[:],
                            start=True,
                            stop=False,
                        )
                        nc.tensor.matmul(
                            ph[:],
                            w1x_s[0:3, h * 128:(h + 1) * 128],
                            ptsT3[:, sc * SC_PTS:(sc + 1) * SC_PTS],
                            start=False,
                            stop=True,
                        )
                        ht = hsb.tile([128, 512], f32, tag=f"h{h}")
                        nc.scalar.activation(
                            ht[:], ph[:], AF.Relu, bias=b1_s[:, h:h + 1]
                        )
                        hts.append(ht)

                    po8 = ps_o8.tile([8, 512], f32)
                    for h in range(4):
                        nc.tensor.matmul(
                            po8[:],
                            w2_s[:, h * 8:(h + 1) * 8],
                            hts[h][:],
                            start=(h == 0),
                            stop=(h == 3),
                        )
                    o8 = o8sb.tile([8, 512], f32)
                    nc.scalar.activation(
                        o8[:], po8[:], AF.Identity, bias=b2_s[:, 0:1]
                    )
                    for c in range(4):
                        cg = sc * 4 + c
                        nc.tensor.transpose(
                            tb[:, 8 * cg:8 * cg + 8],
                            o8[:, c * 128:(c + 1) * 128],
                            id8[:],
                        )

                # ---- postprocess the group (SoA layout, slices [128, W]) ----
                soa = soa_p.tile([128, 8 * W], f32)
                nc.vector.tensor_copy(
                    soa[:].rearrange("p (ch c) -> p c ch", ch=8),
                    tb[:].rearrange("p (c ch) -> p c ch", ch=8),
                )
                soap = soap_p.tile([128, 3 * W], f32)
                nc.vector.tensor_copy(
                    soap[:].rearrange("p (j c) -> p c j", j=3),
                    ptile[:].rearrange("p (c j) -> p c j", j=3),
                )

                S = soa[:, 0:W]
                Z1 = soa[:, W:4 * W]
                Z2 = soa[:, 4 * W:7 * W]
                WW = soa[:, 7 * W:8 * W]

                z1n = scr.tile([128, 5 * W], f32, tag="z1n")
                z2n = scr.tile([128, 5 * W], f32, tag="z2n")
                sq = scr.tile([128, 3 * W], f32, tag="sq")
                t2 = scr.tile([128, 3 * W], f32, tag="t2")
                tr = scr.tile([128, 3 * W], f32, tag="tr")
                nrm = scr.tile([128, W], f32, tag="nrm")
                inv = scr.tile([128, W], f32, tag="inv")
                inner = scr.tile([128, W], f32, tag="inner")
                wc = scr.tile([128, W], f32, tag="wc")
                wr = scr.tile([128, W], f32, tag="wr")

                def bc3(ap):
                    return ap.unsqueeze(1).broadcast_to([128, 3, W])

                def seg3(ap):
                    return ap.rearrange("p (j c) -> p c j", j=3)

                TT = nc.vector.tensor_tensor
                TS = nc.vector.tensor_scalar

                # z1n = z1 / ||z1||
                TT(sq[:], Z1, Z1, ALU.mult)
                nc.vector.tensor_reduce(
                    nrm[:], seg3(sq[:]), axis=mybir.AxisListType.X, op=ALU.add
                )
                nc.scalar.activation(nrm[:], nrm[:], AF.Sqrt)
                nc.vector.reciprocal(inv[:], nrm[:])
                TT(z1n[:, 0:3 * W], Z1, bc3(inv[:]), ALU.mult)
                nc.vector.tensor_copy(z1n[:, 3 * W:5 * W], z1n[:, 0:2 * W])
                # inner = z1n . z2 ; z2o = z2 - inner*z1n ; z2n = z2o/||z2o||
                TT(sq[:], z1n[:, 0:3 * W], Z2, ALU.mult)
                nc.vector.tensor_reduce(
                    inner[:], seg3(sq[:]), axis=mybir.AxisListType.X, op=ALU.add
                )
                TT(t2[:], z1n[:, 0:3 * W], bc3(inner[:]), ALU.mult)
                TT(z2n[:, 0:3 * W], Z2, t2[:], ALU.subtract)
                TT(sq[:], z2n[:, 0:3 * W], z2n[:, 0:3 * W], ALU.mult)
                nc.vector.tensor_reduce(
                    nrm[:], seg3(sq[:]), axis=mybir.AxisListType.X, op=ALU.add
                )
                nc.scalar.activation(nrm[:], nrm[:], AF.Sqrt)
                nc.vector.reciprocal(inv[:], nrm[:])
                TT(z2n[:, 0:3 * W], z2n[:, 0:3 * W], bc3(inv[:]), ALU.mult)
                nc.vector.tensor_copy(z2n[:, 3 * W:5 * W], z2n[:, 0:2 * W])
                # cross = z2n x z1n, normalized  (app x base)
                TT(tr[:], z2n[:, W:4 * W], z1n[:, 2 * W:5 * W], ALU.mult)
                TT(t2[:], z2n[:, 2 * W:5 * W], z1n[:, W:4 * W], ALU.mult)
                TT(tr[:], tr[:], t2[:], ALU.subtract)
                TT(sq[:], tr[:], tr[:], ALU.mult)
                nc.vector.tensor_reduce(
                    nrm[:], seg3(sq[:]), axis=mybir.AxisListType.X, op=ALU.add
                )
                nc.scalar.activation(nrm[:], nrm[:], AF.Sqrt)
                nc.vector.reciprocal(inv[:], nrm[:])
                TT(tr[:], tr[:], bc3(inv[:]), ALU.mult)
                # tr now = grasp_y; move to t2, reuse tr for trans
                nc.vector.tensor_copy(t2[:], tr[:])
                # w clip / relu / half
                TS(wc[:], WW, -0.08, None, ALU.max)
                TS(wc[:], wc[:], 0.08, None, ALU.min)
                TS(wr[:], wc[:], 0.0, None, ALU.max)
                TS(wc[:], wc[:], 0.5, None, ALU.mult)
                # trans = pts - d*app + (w/2)*base
                TS(tr[:], z2n[:, 0:3 * W], -GRIPPER_DEPTH, None, ALU.mult)
                TT(tr[:], tr[:], soap[:], ALU.add)
                TT(sq[:], z1n[:, 0:3 * W], bc3(wc[:]), ALU.mult)
                TT(tr[:], tr[:], sq[:], ALU.add)

                # ---- assemble [128, (c 16)] grasp tile ----
                gout = gout_p.tile([128, 16 * W], f32)
                gview = gout[:].rearrange("p (c i v) -> p i c v", i=4, v=4)

                def gcol(v):
                    # AP [128, 3, W] over grasp rows 0..2, vector column v
                    return gview[:, 0:3, :, v:v + 1].squeeze(3)

                nc.vector.tensor_copy(gcol(0), seg3(z1n[:, 0:3 * W]).transpose([0, 2, 1]))
                nc.vector.tensor_copy(gcol(1), seg3(t2[:]).transpose([0, 2, 1]))
                nc.vector.tensor_copy(gcol(2), seg3(z2n[:, 0:3 * W]).transpose([0, 2, 1]))
                nc.vector.tensor_copy(gcol(3), seg3(tr[:]).transpose([0, 2, 1]))
                nc.vector.memset(gview[:, 3:4, :, 0:3].squeeze(1), 0.0)
                nc.vector.memset(gview[:, 3:4, :, 3:4].squeeze(1), 1.0)

                swt = sw_p.tile([128, 2 * W], f32)
                swv = swt[:].rearrange("p (c q) -> p c q", q=2)
                nc.vector.tensor_copy(swv[:, :, 0:1].squeeze(2), S)
                nc.vector.tensor_copy(swv[:, :, 1:2].squeeze(2), wr[:])

                nc.sync.dma_start(
                    grasps_d[g0:g0 + GROUP_PTS, :].rearrange(
                        "(c p) q -> p c q", p=128
                    ),
                    gout[:].rearrange("p (c q) -> p c q", q=16),
                )
                nc.sync.dma_start(
                    sw_d[g0:g0 + GROUP_PTS, :].rearrange(
                        "(c p) q -> p c q", p=128
                    ),
                    swt[:].rearrange("p (c q) -> p c q", q=2),
                )

    nc.compile()
    return nc


def _pack_weights(inputs):
    w1f = np.concatenate(
        [inputs[f"W1_{n}"][:, 3:].T for n in ("s", "z1", "z2", "w")], axis=1
    ).astype(np.float32)                                   # [128, 512]
    w1x = np.concatenate(
        [inputs[f"W1_{n}"][:, 0:3].T for n in ("s", "z1", "z2", "w")], axis=1
    ).astype(np.float32)                                   # [3, 512]
    w2p = np.zeros((128, 32), np.float32)
    b2p = np.zeros((8, 1), np.float32)
    for h, n in enumerate(("s", "z1", "z2", "w")):
        od, off = HEAD_OD[h], HEAD_OFF[h]
        w2p[:, 8 * h + off:8 * h + off + od] = inputs[f"W2_{n}"].T
        b2p[off:off + od, 0] = inputs[f"b2_{n}"]
    b1p = np.stack(
        [inputs[f"b1_{n}"] for n in ("s", "z1", "z2", "w")], axis=1
    ).astype(np.float32)                                   # [128, 4]
    return w1f, w1x, w2p, b1p, b2p


def kernel(**inputs):
    from concourse.bass_utils import run_bass_kernel_spmd

    inputs = {k: np.asarray(v) for k, v in inputs.items()}
    if "nc" not in _CACHE:
        _CACHE["nc"] = _build_nc()
    nc = _CACHE["nc"]

    point_feat = inputs["point_feat"].astype(np.float32)
    points = inputs["points"].astype(np.float32)
    w1f, w1x, w2p, b1p, b2p = _pack_weights(inputs)
    id128 = np.eye(128, dtype=np.float32)
    id8 = np.eye(8, dtype=np.float32)

    in_maps = []
    for i in range(N_CORES):
        s = slice(i * NC_PTS, (i + 1) * NC_PTS)
        in_maps.append({
            "feat": point_feat[s],
            "pts": points[s],
            "w1f": w1f, "w1x": w1x, "w2p": w2p,
            "b1p": b1p, "b2p": b2p,
            "id128": id128, "id8": id8,
        })

    res = run_bass_kernel_spmd(
        nc, in_maps, list(range(N_CORES)),
        trace=bool(int(os.environ.get("KERNEL_TRACE", "0"))),
    )
    _CACHE["last_result"] = res

    grasps = np.concatenate([r["grasps"] for r in res.results], axis=0)
    sw = np.concatenate([r["sw"] for r in res.results], axis=0)

    B, P = 16, 20480
    pts_out = points.reshape(B, P, 3)
    grasps_out = grasps.reshape(B, P, 4, 4)
    s_out = sw[:, 0].reshape(B, P)
    w_out = sw[:, 1].reshape(B, P)
    feats_out = np.concatenate([points, point_feat], axis=1).reshape(B, P, 131)
    return pts_out, grasps_out, s_out, w_out, feats_out
